# revision 9
# baseline (speedup 1.0000x reference)
"""ChebyKAN layer on 8 TRN2 NeuronCores (data-parallel over batch).

y[b,o] = sum_{i,d} T_d(tanh(x[b,i])) * C[i,o,d],  d = 0..8

Key idea vs the DVE-recurrence baseline: almost the whole Chebyshev basis
is built on the ACT engine with Square ops (1-ULP, present in every ACT
table set -> a single table load), using affine pre-scale folding:

    t   = tanh(x)                     (ACT Tanh)
    q2  = (sqrt2*t)^2        = 2t^2          = T2 + 1
    q4  = (sqrt2*q2-sqrt2)^2 = 2(q2-1)^2     = T4 + 1
    q8  = (sqrt2*q4-sqrt2)^2 = 2(q4-1)^2     = T8 + 1
    r5  = (2*q2-2.5)^2       = 16t^4-20t^2+6.25

The PE consumes 8 basis rows per i-chunk; affine shifts between these
rows and true Chebyshev polynomials are folded into host-side
coefficient/bias transforms (all linear):

    row0 = t          -> C1' = C1 - 3*C3 - 1.25*C5 - C7
    row1 = T2 = q2-1  -> C2
    row2 = m3 = 4t^3  -> C3          (T3 = m3 - 3t)
    row3 = B5 = r5*t  -> C5          (T5 = B5 - 1.25t)
    row4 = B6 = 2T3^2 -> C6          (T6 = B6 - 1)
    row5 = T4 = q4-1  -> C4
    row6 = B7 = 2T4T3 -> C7          (T7 = B7 - t)
    row7 = q8         -> C8          (T8 = q8 - 1)
    bias' = sum_i (C0 - C6 - C8)     (T0 term + the shifted rows)

Per core (batch shard 2048 rows) the work is 4 "quarters" of 512 rows,
each mapping 1:1 onto a PSUM accumulation group of 4 banks; basis tiles
are [i=128 part, b=512 free] fp16, double-buffered so ACT/DVE production
of quarter q+1 overlaps PE consumption of quarter q. DVE does 7 cheap
fp16 products per (quarter, i-chunk); GpSimd evacuates PSUM.

Inputs arrive FULL; sharding/transpose/folding happen on the host here.
"""

import numpy as np

import concourse.bacc as bacc
import concourse.tile as tile
from concourse import mybir
from concourse.bass_utils import run_bass_kernel_spmd

dt = mybir.dt

BATCH = 16384
I_DIM = 512
O_DIM = 512
N_CORES = 8
B_CORE = BATCH // N_CORES      # 2048
QW = 512                       # quarter width (psum group rows)
N_Q = B_CORE // QW             # 4
N_IC = I_DIM // 128            # 4
N_BS = QW // 128               # 4
SQ2 = float(np.float32(np.sqrt(2.0)))

_CACHE = {}


def _build_program():
    from contextlib import ExitStack

    AF = mybir.ActivationFunctionType
    OP = mybir.AluOpType

    nc = bacc.Bacc(num_swdge_queues=4)
    xt_in = nc.declare_dram_parameter("xt", [I_DIM, B_CORE], dt.float32, isOutput=False)
    cd_in = nc.declare_dram_parameter("cd", [8, I_DIM, O_DIM], dt.float32, isOutput=False)
    bias_in = nc.declare_dram_parameter("bias", [1, O_DIM], dt.float32, isOutput=False)
    ones_in = nc.declare_dram_parameter("ones", [1, 128], dt.float32, isOutput=False)
    y_out = nc.declare_dram_parameter("y", [B_CORE, O_DIM], dt.float32, isOutput=True)

    with tile.TileContext(nc) as tc, ExitStack() as ctx:
        cpool = ctx.enter_context(tc.tile_pool(name="cpool", bufs=1))
        xpool = ctx.enter_context(tc.tile_pool(name="xpool", bufs=2))
        fpool = ctx.enter_context(tc.tile_pool(name="fpool", bufs=2))
        bpool = ctx.enter_context(tc.tile_pool(name="bpool", bufs=2))
        tpool = ctx.enter_context(tc.tile_pool(name="tpool", bufs=2))
        epool = ctx.enter_context(tc.tile_pool(name="epool", bufs=4))
        pspool = ctx.enter_context(tc.tile_pool(name="pspool", bufs=8, space="PSUM"))

        # Bias (T_0 + shifted-row corrections) seeded via K=1 matmul.
        bias_t = cpool.tile([1, O_DIM], dt.float16, tag="bias")
        nc.gpsimd.dma_start(out=bias_t[:], in_=bias_in[:])
        ones_t = cpool.tile([1, 128], dt.float16, tag="ones")
        nc.gpsimd.dma_start(out=ones_t[:], in_=ones_in[:])

        # Const bias columns for ACT Square affine pre-scale. Memset on DVE:
        # a gpsimd memset forces a ~3us Q7 drain that delays the coefficient
        # DMA triggers queued behind it.
        nsq2 = cpool.tile([128, 1], dt.float32, tag="nsq2")
        nc.vector.memset(nsq2[:], -SQ2)
        n2p5 = cpool.tile([128, 1], dt.float32, tag="n2p5")
        nc.vector.memset(n2p5[:], -2.5)

        # Coefficients (host pre-folded/ordered): cast-DMA fp32 -> fp16.
        c_tiles = []
        for r in range(8):
            c = cpool.tile([128, N_IC, O_DIM], dt.float16, tag=f"c{r}", name=f"c{r}")
            nc.gpsimd.dma_start(
                out=c[:],
                in_=cd_in[r].rearrange("(ic p) o -> p ic o", p=128),
            )
            c_tiles.append(c)

        # Bias broadcast [128, 512]: one K=1 matmul replicates the bias row
        # across partitions; evacuation then fuses the add, so the 4 PSUM
        # seeds per quarter (16 matmuls) are not needed.
        bps = pspool.tile([128, O_DIM], dt.float32, tag="ps", name="bps")
        nc.tensor.matmul(bps[:], lhsT=ones_t[:], rhs=bias_t[:], start=True, stop=True)
        bias_bc = cpool.tile([128, O_DIM], dt.float32, tag="bias_bc")
        nc.vector.tensor_copy(bias_bc[:], bps[:])

        for q in range(N_Q):
            b0 = q * QW

            # ---- basis production (type-major across i-chunks) ----
            xts = []
            for ic in range(N_IC):
                xt = xpool.tile([128, QW], dt.float32, tag=f"x{ic}")
                nc.sync.dma_start(
                    out=xt[:], in_=xt_in[ic * 128:(ic + 1) * 128, b0:b0 + QW]
                )
                xts.append(xt)

            rows = [[None] * N_IC for _ in range(8)]

            # ACT stream (one table set: tanh + square live together)
            t16s, t32s, q2s, r5s, q4s = [], [], [], [], []
            for ic in range(N_IC):
                t32 = fpool.tile([128, QW], dt.float32, tag=f"t32_{ic}", name="t32")
                nc.scalar.activation(t32[:], xts[ic][:], AF.Tanh)
                t32s.append(t32)
            # t16 (PE row 0) via GpSimd cast-copy: keeps ACT free and makes
            # row 0 available right behind t32 instead of after a second
            # tanh pass (the scheduler runs the deep-chain t32s first).
            for ic in range(N_IC):
                t16 = bpool.tile([128, QW], dt.float16, tag=f"b0_{ic}", name="t16")
                nc.gpsimd.tensor_copy(t16[:], t32s[ic][:])
                rows[0][ic] = t16
                t16s.append(t16)
            for ic in range(N_IC):
                q2 = fpool.tile([128, QW], dt.float32, tag=f"q2_{ic}", name="q2")
                nc.scalar.activation(q2[:], t32s[ic][:], AF.Square, scale=SQ2)
                q2s.append(q2)
            for ic in range(N_IC):
                r5 = tpool.tile([128, QW], dt.float16, tag=f"r5_{ic}", name="r5")
                nc.scalar.activation(
                    r5[:], q2s[ic][:], AF.Square, bias=n2p5[:], scale=2.0
                )
                r5s.append(r5)
            for ic in range(N_IC):
                q4 = fpool.tile([128, QW], dt.float32, tag=f"q4_{ic}", name="q4")
                nc.scalar.activation(
                    q4[:], q2s[ic][:], AF.Square, bias=nsq2[:], scale=SQ2
                )
                q4s.append(q4)
            for ic in range(N_IC):
                q8 = bpool.tile([128, QW], dt.float16, tag=f"b7_{ic}", name="q8")
                nc.scalar.activation(
                    q8[:], q4s[ic][:], AF.Square, bias=nsq2[:], scale=SQ2
                )
                rows[7][ic] = q8

            # DVE stream (fp16 products; order matches ACT production)
            for ic in range(N_IC):
                T2 = bpool.tile([128, QW], dt.float16, tag=f"b1_{ic}", name="T2")
                nc.vector.tensor_scalar_sub(T2[:], q2s[ic][:], 1.0)
                rows[1][ic] = T2
            m3s = []
            for ic in range(N_IC):
                m3 = bpool.tile([128, QW], dt.float16, tag=f"b2_{ic}", name="m3")
                nc.vector.scalar_tensor_tensor(
                    m3[:], q2s[ic][:], 2.0, t32s[ic][:], OP.mult, OP.mult
                )
                rows[2][ic] = m3
                m3s.append(m3)
            T3s = []
            for ic in range(N_IC):
                T3 = tpool.tile([128, QW], dt.float16, tag=f"t3_{ic}", name="T3")
                nc.vector.scalar_tensor_tensor(
                    T3[:], t16s[ic][:], -3.0, m3s[ic][:], OP.mult, OP.add
                )
                T3s.append(T3)
            for ic in range(N_IC):
                B5 = bpool.tile([128, QW], dt.float16, tag=f"b3_{ic}", name="B5")
                nc.vector.tensor_mul(B5[:], r5s[ic][:], t16s[ic][:])
                rows[3][ic] = B5
            for ic in range(N_IC):
                B6 = bpool.tile([128, QW], dt.float16, tag=f"b4_{ic}", name="B6")
                nc.vector.scalar_tensor_tensor(
                    B6[:], T3s[ic][:], 2.0, T3s[ic][:], OP.mult, OP.mult
                )
                rows[4][ic] = B6
            T4s = []
            for ic in range(N_IC):
                T4 = bpool.tile([128, QW], dt.float16, tag=f"b5_{ic}", name="T4")
                nc.vector.tensor_scalar_sub(T4[:], q4s[ic][:], 1.0)
                rows[5][ic] = T4
                T4s.append(T4)
            for ic in range(N_IC):
                B7 = bpool.tile([128, QW], dt.float16, tag=f"b6_{ic}", name="B7")
                nc.vector.scalar_tensor_tensor(
                    B7[:], T4s[ic][:], 2.0, T3s[ic][:], OP.mult, OP.mult
                )
                rows[6][ic] = B7

            # ---- PE: accumulate 8 rows x 4 i-chunks per PSUM bank ----
            ps = []
            for bs in range(N_BS):
                p = pspool.tile([128, O_DIM], dt.float32, tag="ps", name="ps")
                ps.append(p)
            for r in range(8):
                for ic in range(N_IC):
                    lt = rows[r][ic]
                    for bs in range(N_BS):
                        nc.tensor.matmul(
                            ps[bs][:],
                            lhsT=lt[:, bs * 128:(bs + 1) * 128],
                            rhs=c_tiles[r][:, ic, :],
                            start=(r == 0 and ic == 0),
                            stop=(r == 7 and ic == N_IC - 1),
                        )

            # ---- evacuate PSUM (DVE, fusing the bias add) and store ----
            for bs in range(N_BS):
                e = epool.tile([128, O_DIM], dt.float32, tag="e")
                nc.vector.tensor_add(e[:], ps[bs][:], bias_bc[:])
                nc.sync.dma_start(
                    out=y_out[b0 + bs * 128: b0 + (bs + 1) * 128, :], in_=e[:]
                )

    nc.compile()
    return nc


def _get_program():
    if "nc" not in _CACHE:
        _CACHE["nc"] = _build_program()
    return _CACHE["nc"]


def _prep_inputs(x, cheby_coeffs):
    x = np.ascontiguousarray(x, dtype=np.float32)
    C = np.asarray(cheby_coeffs, dtype=np.float64)  # [I, O, 9]
    # Host-side folding of affine shifts between device rows and T_d.
    c1p = C[:, :, 1] - 3.0 * C[:, :, 3] - 1.25 * C[:, :, 5] - C[:, :, 7]
    rows = [c1p, C[:, :, 2], C[:, :, 3], C[:, :, 5],
            C[:, :, 6], C[:, :, 4], C[:, :, 7], C[:, :, 8]]
    cd = np.ascontiguousarray(np.stack(rows, axis=0), dtype=np.float32)
    bias = (C[:, :, 0].sum(axis=0) - C[:, :, 6].sum(axis=0)
            - C[:, :, 8].sum(axis=0)).astype(np.float32)[None, :]
    ones = np.ones((1, 128), dtype=np.float32)
    in_maps = []
    for core in range(N_CORES):
        xs = x[core * B_CORE:(core + 1) * B_CORE]          # [2048, I]
        xt = np.ascontiguousarray(xs.T)                     # [I, 2048]
        in_maps.append({"xt": xt, "cd": cd, "bias": bias, "ones": ones})
    return in_maps


def run(x, cheby_coeffs, trace=False, **trace_kwargs):
    nc = _get_program()
    in_maps = _prep_inputs(x, cheby_coeffs)
    res = run_bass_kernel_spmd(
        nc, in_maps, list(range(N_CORES)), trace=trace, **trace_kwargs
    )
    y = np.concatenate([res.results[i]["y"] for i in range(N_CORES)], axis=0)
    return y, res


def kernel(x, cheby_coeffs):
    y, _ = run(x, cheby_coeffs)
    return y


# revision 14
# speedup vs baseline: 1.0524x; 1.0524x over previous
"""ChebyKAN layer on 8 TRN2 NeuronCores (data-parallel over batch).

y[b,o] = sum_{i,d} T_d(tanh(x[b,i])) * C[i,o,d],  d = 0..8

Key idea vs the DVE-recurrence baseline: almost the whole Chebyshev basis
is built on the ACT engine with Square ops (1-ULP, present in every ACT
table set -> a single table load), using affine pre-scale folding:

    t   = tanh(x)                     (ACT Tanh)
    q2  = (sqrt2*t)^2        = 2t^2          = T2 + 1
    q4  = (sqrt2*q2-sqrt2)^2 = 2(q2-1)^2     = T4 + 1
    q8  = (sqrt2*q4-sqrt2)^2 = 2(q4-1)^2     = T8 + 1
    r5  = (2*q2-2.5)^2       = 16t^4-20t^2+6.25

The PE consumes 8 basis rows per i-chunk; affine shifts between these
rows and true Chebyshev polynomials are folded into host-side
coefficient/bias transforms (all linear):

    row0 = t          -> C1' = C1 - 3*C3 - 1.25*C5 - C7
    row1 = T2 = q2-1  -> C2
    row2 = m3 = 4t^3  -> C3          (T3 = m3 - 3t)
    row3 = B5 = r5*t  -> C5          (T5 = B5 - 1.25t)
    row4 = B6 = 2T3^2 -> C6          (T6 = B6 - 1)
    row5 = T4 = q4-1  -> C4
    row6 = B7 = 2T4T3 -> C7          (T7 = B7 - t)
    row7 = q8         -> C8          (T8 = q8 - 1)
    bias' = sum_i (C0 - C6 - C8)     (T0 term + the shifted rows)

Per core (batch shard 2048 rows) the work is 4 "quarters" of 512 rows,
each mapping 1:1 onto a PSUM accumulation group of 4 banks; basis tiles
are [i=128 part, b=512 free] fp16, double-buffered so ACT/DVE production
of quarter q+1 overlaps PE consumption of quarter q. DVE does 7 cheap
fp16 products per (quarter, i-chunk); GpSimd evacuates PSUM.

Inputs arrive FULL; sharding/transpose/folding happen on the host here.
"""

import numpy as np

import concourse.bacc as bacc
import concourse.tile as tile
from concourse import mybir
from concourse.bass_utils import run_bass_kernel_spmd

dt = mybir.dt

BATCH = 16384
I_DIM = 512
O_DIM = 512
N_CORES = 8
B_CORE = BATCH // N_CORES      # 2048
QW = 512                       # quarter width (psum group rows)
N_Q = B_CORE // QW             # 4
N_IC = I_DIM // 128            # 4
N_BS = QW // 128               # 4
SQ2 = float(np.float32(np.sqrt(2.0)))

_CACHE = {}


def _build_program():
    from contextlib import ExitStack

    AF = mybir.ActivationFunctionType
    OP = mybir.AluOpType

    nc = bacc.Bacc(num_swdge_queues=4)
    xt_in = nc.declare_dram_parameter("xt", [I_DIM, B_CORE], dt.float16, isOutput=False)
    cd_in = nc.declare_dram_parameter("cd", [8, I_DIM, O_DIM], dt.float16, isOutput=False)
    bias_in = nc.declare_dram_parameter("bias", [1, O_DIM], dt.float16, isOutput=False)
    ones_in = nc.declare_dram_parameter("ones", [1, 128], dt.float16, isOutput=False)
    y_out = nc.declare_dram_parameter("y", [B_CORE, O_DIM], dt.float32, isOutput=True)

    with tile.TileContext(nc) as tc, ExitStack() as ctx:
        cpool = ctx.enter_context(tc.tile_pool(name="cpool", bufs=1))
        xpool = ctx.enter_context(tc.tile_pool(name="xpool", bufs=2))
        fpool = ctx.enter_context(tc.tile_pool(name="fpool", bufs=2))
        bpool = ctx.enter_context(tc.tile_pool(name="bpool", bufs=2))
        tpool = ctx.enter_context(tc.tile_pool(name="tpool", bufs=2))
        epool = ctx.enter_context(tc.tile_pool(name="epool", bufs=4))
        pspool = ctx.enter_context(tc.tile_pool(name="pspool", bufs=8, space="PSUM"))

        # Bias (T_0 + shifted-row corrections) seeded via K=1 matmul.
        bias_t = cpool.tile([1, O_DIM], dt.float16, tag="bias")
        nc.gpsimd.dma_start(out=bias_t[:], in_=bias_in[:])
        ones_t = cpool.tile([1, 128], dt.float16, tag="ones")
        nc.gpsimd.dma_start(out=ones_t[:], in_=ones_in[:])

        # Const bias columns for ACT Square affine pre-scale. Memset on DVE:
        # a gpsimd memset forces a ~3us Q7 drain that delays the coefficient
        # DMA triggers queued behind it.
        nsq2 = cpool.tile([128, 1], dt.float32, tag="nsq2")
        nc.vector.memset(nsq2[:], -SQ2)
        n2p5 = cpool.tile([128, 1], dt.float32, tag="n2p5")
        nc.vector.memset(n2p5[:], -2.5)

        # Coefficients (host pre-folded/ordered/cast to fp16). Trigger on the
        # Activation HW-DGE queue: the gpsimd SW queue serializes all eight
        # 0.5MB reads behind one ring, which starved the first matmuls.
        c_tiles = []
        for r in range(8):
            c = cpool.tile([128, N_IC, O_DIM], dt.float16, tag=f"c{r}", name=f"c{r}")
            nc.scalar.dma_start(
                out=c[:],
                in_=cd_in[r].rearrange("(ic p) o -> p ic o", p=128),
            )
            c_tiles.append(c)

        # Bias broadcast [128, 512]: one K=1 matmul replicates the bias row
        # across partitions; evacuation then fuses the add, so the 4 PSUM
        # seeds per quarter (16 matmuls) are not needed.
        bps = pspool.tile([128, O_DIM], dt.float32, tag="ps", name="bps")
        nc.tensor.matmul(bps[:], lhsT=ones_t[:], rhs=bias_t[:], start=True, stop=True)
        bias_bc = cpool.tile([128, O_DIM], dt.float32, tag="bias_bc")
        nc.vector.tensor_copy(bias_bc[:], bps[:])

        for q in range(N_Q):
            b0 = q * QW

            # ---- basis production (type-major across i-chunks) ----
            xts = []
            for ic in range(N_IC):
                xt = xpool.tile([128, QW], dt.float16, tag=f"x{ic}")
                nc.sync.dma_start(
                    out=xt[:], in_=xt_in[ic * 128:(ic + 1) * 128, b0:b0 + QW]
                )
                xts.append(xt)

            rows = [[None] * N_IC for _ in range(8)]

            # ACT stream (one table set: tanh + square live together)
            t16s, t32s, q2s, r5s, q4s = [], [], [], [], []
            for ic in range(N_IC):
                t32 = fpool.tile([128, QW], dt.float32, tag=f"t32_{ic}", name="t32")
                nc.scalar.activation(t32[:], xts[ic][:], AF.Tanh)
                t32s.append(t32)
            # t16 (PE row 0) via DVE cast-copy: DVE is idle this early, so
            # row 0 lands right behind t32 instead of after a second tanh
            # pass (the scheduler runs the deep-chain t32s first).
            for ic in range(N_IC):
                t16 = bpool.tile([128, QW], dt.float16, tag=f"b0_{ic}", name="t16")
                nc.vector.tensor_copy(t16[:], t32s[ic][:])
                rows[0][ic] = t16
                t16s.append(t16)
            for ic in range(N_IC):
                q2 = fpool.tile([128, QW], dt.float32, tag=f"q2_{ic}", name="q2")
                nc.scalar.activation(q2[:], t32s[ic][:], AF.Square, scale=SQ2)
                q2s.append(q2)
            for ic in range(N_IC):
                r5 = tpool.tile([128, QW], dt.float16, tag=f"r5_{ic}", name="r5")
                nc.scalar.activation(
                    r5[:], q2s[ic][:], AF.Square, bias=n2p5[:], scale=2.0
                )
                r5s.append(r5)
            for ic in range(N_IC):
                q4 = fpool.tile([128, QW], dt.float32, tag=f"q4_{ic}", name="q4")
                nc.scalar.activation(
                    q4[:], q2s[ic][:], AF.Square, bias=nsq2[:], scale=SQ2
                )
                q4s.append(q4)
            for ic in range(N_IC):
                q8 = bpool.tile([128, QW], dt.float16, tag=f"b7_{ic}", name="q8")
                nc.scalar.activation(
                    q8[:], q4s[ic][:], AF.Square, bias=nsq2[:], scale=SQ2
                )
                rows[7][ic] = q8

            # DVE stream (fp16 products; order matches ACT production)
            for ic in range(N_IC):
                T2 = bpool.tile([128, QW], dt.float16, tag=f"b1_{ic}", name="T2")
                nc.vector.tensor_scalar_sub(T2[:], q2s[ic][:], 1.0)
                rows[1][ic] = T2
            m3s = []
            for ic in range(N_IC):
                m3 = bpool.tile([128, QW], dt.float16, tag=f"b2_{ic}", name="m3")
                nc.vector.scalar_tensor_tensor(
                    m3[:], q2s[ic][:], 2.0, t32s[ic][:], OP.mult, OP.mult
                )
                rows[2][ic] = m3
                m3s.append(m3)
            T3s = []
            for ic in range(N_IC):
                T3 = tpool.tile([128, QW], dt.float16, tag=f"t3_{ic}", name="T3")
                nc.vector.scalar_tensor_tensor(
                    T3[:], t16s[ic][:], -3.0, m3s[ic][:], OP.mult, OP.add
                )
                T3s.append(T3)
            for ic in range(N_IC):
                B5 = bpool.tile([128, QW], dt.float16, tag=f"b3_{ic}", name="B5")
                nc.vector.tensor_mul(B5[:], r5s[ic][:], t16s[ic][:])
                rows[3][ic] = B5
            for ic in range(N_IC):
                B6 = bpool.tile([128, QW], dt.float16, tag=f"b4_{ic}", name="B6")
                nc.vector.scalar_tensor_tensor(
                    B6[:], T3s[ic][:], 2.0, T3s[ic][:], OP.mult, OP.mult
                )
                rows[4][ic] = B6
            T4s = []
            for ic in range(N_IC):
                T4 = bpool.tile([128, QW], dt.float16, tag=f"b5_{ic}", name="T4")
                nc.vector.tensor_scalar_sub(T4[:], q4s[ic][:], 1.0)
                rows[5][ic] = T4
                T4s.append(T4)
            for ic in range(N_IC):
                B7 = bpool.tile([128, QW], dt.float16, tag=f"b6_{ic}", name="B7")
                nc.vector.scalar_tensor_tensor(
                    B7[:], T4s[ic][:], 2.0, T3s[ic][:], OP.mult, OP.mult
                )
                rows[6][ic] = B7

            # ---- PE: accumulate 8 rows x 4 i-chunks per PSUM bank ----
            ps = []
            for bs in range(N_BS):
                p = pspool.tile([128, O_DIM], dt.float32, tag="ps", name="ps")
                ps.append(p)
            for r in range(8):
                for ic in range(N_IC):
                    lt = rows[r][ic]
                    for bs in range(N_BS):
                        nc.tensor.matmul(
                            ps[bs][:],
                            lhsT=lt[:, bs * 128:(bs + 1) * 128],
                            rhs=c_tiles[r][:, ic, :],
                            start=(r == 0 and ic == 0),
                            stop=(r == 7 and ic == N_IC - 1),
                        )

            # ---- evacuate PSUM (DVE, fusing the bias add) and store ----
            for bs in range(N_BS):
                e = epool.tile([128, O_DIM], dt.float32, tag="e")
                nc.vector.tensor_add(e[:], ps[bs][:], bias_bc[:])
                nc.sync.dma_start(
                    out=y_out[b0 + bs * 128: b0 + (bs + 1) * 128, :], in_=e[:]
                )

    nc.compile()
    return nc


def _get_program():
    if "nc" not in _CACHE:
        _CACHE["nc"] = _build_program()
    return _CACHE["nc"]


def _prep_inputs(x, cheby_coeffs):
    x = np.asarray(x, dtype=np.float32)
    C = np.asarray(cheby_coeffs, dtype=np.float64)  # [I, O, 9]
    # Host-side folding of affine shifts between device rows and T_d.
    c1p = C[:, :, 1] - 3.0 * C[:, :, 3] - 1.25 * C[:, :, 5] - C[:, :, 7]
    rows = [c1p, C[:, :, 2], C[:, :, 3], C[:, :, 5],
            C[:, :, 6], C[:, :, 4], C[:, :, 7], C[:, :, 8]]
    cd = np.ascontiguousarray(np.stack(rows, axis=0), dtype=np.float16)
    bias = (C[:, :, 0].sum(axis=0) - C[:, :, 6].sum(axis=0)
            - C[:, :, 8].sum(axis=0)).astype(np.float16)[None, :]
    ones = np.ones((1, 128), dtype=np.float16)
    xt_all = np.ascontiguousarray(x.T, dtype=np.float16)    # [I, BATCH]
    in_maps = []
    for core in range(N_CORES):
        xt = np.ascontiguousarray(
            xt_all[:, core * B_CORE:(core + 1) * B_CORE])   # [I, 2048]
        in_maps.append({"xt": xt, "cd": cd, "bias": bias, "ones": ones})
    return in_maps


def run(x, cheby_coeffs, trace=False, **trace_kwargs):
    nc = _get_program()
    in_maps = _prep_inputs(x, cheby_coeffs)
    res = run_bass_kernel_spmd(
        nc, in_maps, list(range(N_CORES)), trace=trace, **trace_kwargs
    )
    y = np.concatenate([res.results[i]["y"] for i in range(N_CORES)], axis=0)
    return y, res


def kernel(x, cheby_coeffs):
    y, _ = run(x, cheby_coeffs)
    return y
